# revision 1
# baseline (speedup 1.0000x reference)
"""LIF spike kernel for Trainium2 (Bass/Tile), data-parallel over 8 NeuronCores.

Problem: x [32, 8, 128, 32, 32] fp32 -> spikes [32, 8, 128, 32, 32] fp32
    mem_t = mem_{t-1} * 0.25 + x_t ; spike = (mem >= 0.5) ; mem *= (1 - spike)

Sharding: batch dim (32) split 4-per-core across 8 cores; no cross-core comm.

Per-core device program (shapes [4, 8, 128, 1024]):
  - layout: partitions = channel dim C=128, free = (b, h*w) = 4096
  - per time step on VectorE:
        u   = (r * TAU) + x_t            scalar_tensor_tensor, fp32
        y_t = (u >= 0.5)                 tensor_scalar -> uint8 {0,1}
        r   = (u < 0.5) * u              scalar_tensor_tensor (reset)
  - spike output is uint8; host casts back to fp32 (exact, spikes are 0/1).
All arithmetic is fp32 and rounds identically to the jax reference
(mult by 0.25 is exact; a single rounding add per step), so the spike
train is expected to match bitwise.
"""

import os
import numpy as np

B, T, C, H, W = 32, 8, 128, 32, 32
HW = H * W
N_CORES = 8
BPC = B // N_CORES  # batches per core
TAU = 0.25
THRESH = 0.5

_nc_cache = {}
LAST_RESULTS = None


def build_bass(free_w=HW, use_act=False, reps=1):
    """Build the per-core Bass program. free_w lets tests shrink the spatial
    dim for fast simulation; reps>1 repeats the whole computation for
    loop-delta hardware timing (outputs are rewritten identically)."""
    import concourse.bacc as bacc
    import concourse.mybir as mybir
    from concourse.tile import TileContext

    f32 = mybir.dt.float32
    u8 = mybir.dt.uint8
    Alu = mybir.AluOpType

    nc = bacc.Bacc("TRN2", target_bir_lowering=False)
    x = nc.dram_tensor("x", [BPC, T, C, free_w], f32, kind="ExternalInput")
    y = nc.dram_tensor("y", [BPC, T, C, free_w], u8, kind="ExternalOutput")

    with TileContext(nc) as tc:
        with (
            tc.tile_pool(name="xp", bufs=6) as xp,
            tc.tile_pool(name="up", bufs=2) as up,
            tc.tile_pool(name="rp", bufs=2) as rp,
            tc.tile_pool(name="yp", bufs=3) as yp,
            tc.tile_pool(name="cp", bufs=1) as cp,
        ):
            neg_thresh = None
            if use_act:
                neg_thresh = cp.tile([C, 1], f32)
                nc.vector.memset(neg_thresh[:], -THRESH)
            for _rep in range(reps):
                r = None
                for t in range(T):
                    xt = xp.tile([C, BPC, free_w], f32)
                    nc.sync.dma_start(xt[:], x[:, t, :, :].rearrange("b c w -> c b w"))
                    if t == 0:
                        u = xt
                    else:
                        u = up.tile([C, BPC, free_w], f32)
                        nc.vector.scalar_tensor_tensor(
                            u[:], r[:], TAU, xt[:], Alu.mult, Alu.add
                        )
                    yt = yp.tile([C, BPC, free_w], u8)
                    if use_act:
                        # spike = Sign(u - 0.5) saturated to uint8: {-1,0,+1}->{0,0,1}
                        nc.scalar.activation(
                            yt[:],
                            u[:],
                            mybir.ActivationFunctionType.Sign,
                            bias=neg_thresh[:],
                        )
                    else:
                        nc.vector.tensor_scalar(yt[:], u[:], THRESH, None, Alu.is_ge)
                    if t < T - 1:
                        rn = rp.tile([C, BPC, free_w], f32)
                        nc.vector.scalar_tensor_tensor(
                            rn[:], u[:], THRESH, u[:], Alu.is_lt, Alu.mult
                        )
                        r = rn
                    # out-DMAs ride the second HWDGE ring (ACT) so they don't
                    # serialize behind the x loads on the SP ring
                    nc.scalar.dma_start(
                        y[:, t, :, :].rearrange("b c w -> c b w"), yt[:]
                    )
    nc.compile()
    return nc


def build_bass_pe(free_w=HW, reps=1, h_dt="float8e4", chunk=2048):
    """PE variant: per step t>=1, u = 0.125*I @ d + I @ x accumulated in PSUM
    (two diagonal fp32 matmuls per 512-col bank); ACT computes
    h = Sign(0.5 - u) in {+1,0,-1} (doubles as the spike output: spike iff
    h <= 0); DVE computes d = (h + 1) * u = 2*u*[u<0.5] in one fused op.
    The 2x in d is folded into the 0.125 weight (0.25/2)."""
    import concourse.bacc as bacc
    import concourse.mybir as mybir
    from concourse.tile import TileContext

    f32 = mybir.dt.float32
    Alu = mybir.AluOpType
    hdt = getattr(mybir.dt, h_dt)

    nc = bacc.Bacc("TRN2", target_bir_lowering=False)
    x = nc.dram_tensor("x", [BPC, T, C, free_w], f32, kind="ExternalInput")
    y = nc.dram_tensor("y", [BPC, T, C, free_w], hdt, kind="ExternalOutput")
    w8_d = nc.inline_tensor((np.eye(C) * (TAU / 2.0)).astype(np.float32), "w8")
    wid_d = nc.inline_tensor(np.eye(C, dtype=np.float32), "wid")

    FREE = BPC * free_w
    NCH = max(1, FREE // chunk)
    CH = FREE // NCH

    with TileContext(nc) as tc:
        with (
            tc.tile_pool(name="xp", bufs=3) as xp,
            tc.tile_pool(name="dp", bufs=2) as dp,
            tc.tile_pool(name="hp", bufs=3) as hp,
            tc.tile_pool(name="wp", bufs=1) as wp,
            tc.tile_pool(name="cp", bufs=1) as cp,
            tc.tile_pool(name="ps", bufs=2, space="PSUM") as ps,
        ):
            w8 = wp.tile([C, C], f32, tag="w8")
            wid = wp.tile([C, C], f32, tag="wid")
            nc.sync.dma_start(w8[:], w8_d[:])
            nc.sync.dma_start(wid[:], wid_d[:])
            half = cp.tile([C, 1], f32)
            nc.vector.memset(half[:], THRESH)
            for _rep in range(reps):
                d_prev = None
                for t in range(T):
                    xt = xp.tile([C, FREE], f32)
                    nc.sync.dma_start(
                        xt[:].rearrange("c (b w) -> c b w", b=BPC),
                        x[:, t, :, :].rearrange("b c w -> c b w"),
                    )
                    ht = hp.tile([C, FREE], hdt)
                    if t == 0:
                        # u_0 = x_0 lives in SBUF
                        nc.scalar.activation(
                            ht[:], xt[:], mybir.ActivationFunctionType.Sign,
                            bias=half[:], scale=-1.0,
                        )
                        if t < T - 1:
                            dn = dp.tile([C, FREE], f32, tag="d")
                            nc.vector.scalar_tensor_tensor(
                                dn[:], ht[:], 1.0, xt[:], Alu.add, Alu.mult
                            )
                            d_prev = dn
                    else:
                        if t < T - 1:
                            dn = dp.tile([C, FREE], f32, tag="d")
                        else:
                            dn = None
                        for j in range(NCH):
                            sl = slice(j * CH, (j + 1) * CH)
                            pt = ps.tile([C, CH], f32)
                            # matmul output is capped at one PSUM bank
                            # (512 fp32) — slice the psum tile bank-aligned
                            mmw = min(512, CH)
                            for k in range(0, CH, mmw):
                                kk = slice(k, k + mmw)
                                gsl = slice(j * CH + k, j * CH + k + mmw)
                                nc.tensor.matmul(
                                    pt[:, kk], w8[:], d_prev[:, gsl],
                                    start=True, stop=False,
                                )
                                nc.tensor.matmul(
                                    pt[:, kk], wid[:], xt[:, gsl],
                                    start=False, stop=True,
                                )
                            nc.scalar.activation(
                                ht[:, sl], pt[:],
                                mybir.ActivationFunctionType.Sign,
                                bias=half[:], scale=-1.0,
                            )
                            if dn is not None:
                                nc.vector.scalar_tensor_tensor(
                                    dn[:, sl], ht[:, sl], 1.0, pt[:],
                                    Alu.add, Alu.mult,
                                )
                        d_prev = dn
                    # second HWDGE ring (ACT) for stores, SP ring for loads
                    nc.scalar.dma_start(
                        y[:, t, :, :].rearrange("b c w -> c b w"),
                        ht[:].rearrange("c (b w) -> c b w", b=BPC),
                    )
    nc.compile()
    return nc


def _get_nc():
    variant = os.environ.get("LIF_VARIANT", "act")
    key = (HW, variant)
    if key not in _nc_cache:
        if variant == "pe":
            _nc_cache[key] = build_bass_pe(HW)
        else:
            _nc_cache[key] = build_bass(HW, use_act=variant == "act")
    return _nc_cache[key]


def kernel(x):
    global LAST_RESULTS
    from concourse import bass_utils

    assert x.shape == (B, T, C, H, W) and x.dtype == np.float32
    xs = np.ascontiguousarray(x.reshape(B, T, C, HW))
    nc = _get_nc()
    in_maps = [
        {"x": np.ascontiguousarray(xs[i * BPC : (i + 1) * BPC])}
        for i in range(N_CORES)
    ]
    res = bass_utils.run_bass_kernel_spmd(
        nc,
        in_maps,
        core_ids=list(range(N_CORES)),
        trace=bool(int(os.environ.get("LIF_TRACE", "0"))),
    )
    LAST_RESULTS = res
    variant = os.environ.get("LIF_VARIANT", "act")
    out = np.empty((B, T, C, HW), dtype=np.float32)
    for i in range(N_CORES):
        yi = res.results[i]["y"]
        if variant == "pe":
            # h = Sign(0.5-u) in fp8: +1 -> no spike; 0/-1 -> spike
            out[i * BPC : (i + 1) * BPC] = yi.astype(np.float32) < 0.5
        else:
            # spike iff raw uint8 == 1 (DVE is_ge gives {0,1}; ACT Sign gives
            # {-1,0,+1} which lands as {255/0, 0, 1} in uint8 depending on
            # wrap-vs-saturate — spike==1 holds in every case).
            out[i * BPC : (i + 1) * BPC] = yi == 1
    return out.reshape(B, T, C, H, W)



# revision 6
# speedup vs baseline: 18.6380x; 18.6380x over previous
"""LIF spike kernel for Trainium2 (Bass/Tile), data-parallel over 8 NeuronCores.

Problem: x [32, 8, 128, 32, 32] fp32 -> spikes [32, 8, 128, 32, 32] fp32
    mem_t = mem_{t-1} * 0.25 + x_t ; spike = (mem >= 0.5) ; mem *= (1 - spike)

Sharding: batch dim (32) split 4-per-core across 8 cores; no cross-core comm.

Per-core device program (x shaped [4, 8, 128, 1024] fp32, y uint8):
  - The LIF update is elementwise, so no transpose is needed anywhere.
    Per batch b, ONE 4 MiB dma_start loads x[b] into an SBUF tile
    [128, 8*1024] via "t c w -> c (t w)" — per-partition chunks are the
    contiguous 4 KiB w-rows, so the DMA runs near peak HBM bandwidth
    (the old "b c w -> c b w" per-step loads were the documented slow
    rearrange path and dominated the runtime).
  - per time step t on [128, 1024] slices:
        u_t = select(u_{t-1} < 0.5, TAU*u_{t-1}, 0) + x_t
              -- ONE fused custom-DVE op (registered below): decay,
                 reset and input accumulation in a single DVE pass.
        y_t = Sign(u_t - 0.5) on ACT -> uint8 saturates {-1,0,1}->{0,0,1}
  - spikes accumulate in a [128, 8*1024] uint8 tile; ONE 1 MiB store per b.
  - host maps uint8 (spike == 1) back to fp32 (exact).
All arithmetic is fp32 and rounds identically to the jax reference
(mult by 0.25 is exact; select/compare exact; a single rounding add per
step), so the spike train matches bitwise.
"""

import os
import numpy as np

B, T, C, H, W = 32, 8, 128, 32, 32
HW = H * W
N_CORES = 8
BPC = B // N_CORES  # batches per core
TAU = 0.25
THRESH = 0.5

_nc_cache = {}
LAST_RESULTS = None
_LIF_OP = None


def _register_lif_op():
    """Register the fused LIF-step custom DVE op with concourse's runtime
    table (the documented extension point is appending to dve_ops.OPS).

        out = select(in0 < s0, in0 * s1, 0) + in1
            = TAU*u*[u < THRESH] + x      (s0=THRESH, s1=TAU)

    One DVE instruction per time step instead of two scalar_tensor_tensor
    passes; exact fp32 (mult by 2^-2 exact, one rounding add)."""
    global _LIF_OP
    if _LIF_OP is not None:
        return _LIF_OP
    from concourse import dve_ops
    from concourse.dve_spec import (
        Spec,
        Src0,
        Src1,
        C0,
        C1,
        Zero,
        select,
        lower,
        _has_src1,
    )
    from concourse.dve_uop import DveOpSpec

    name = "LIF_STEP_ANT"
    for op in dve_ops.OPS:
        if op.name == name:
            _LIF_OP = op
            return op

    body = select(Src0 < C0, Src0 * C1, Zero) + Src1

    def _ref(in0, in1, s0, s1, imm2):
        return (
            np.where(in0 < s0, in0.astype(np.float32) * s1, 0.0).astype(np.float32)
            + in1
        )

    spec = Spec(body=body, reference=_ref)
    opcode = dve_ops._CUSTOM_DVE_ROW_BASE + len(dve_ops.OPS)
    assert opcode < 0x20
    shas = {}
    for ver in ("v3", "v4"):
        try:
            uops = lower(spec, ver=ver)
        except Exception:
            continue
        shas[ver] = DveOpSpec(
            name=name, opcode=opcode, uops=uops, rd1_en=_has_src1(spec)
        ).sha(ver)
    op = dve_ops.DveOp(name, spec, subdim=False, uops_sha=shas)
    dve_ops.OPS.append(op)
    dve_ops._SUB_OPCODE_FOR_NAME[name] = opcode
    dve_ops.CUSTOM_DVE_SPECS[name] = spec
    _LIF_OP = op
    return op


def build_bass(reps=1, fused=True):
    """Per-core Bass program. reps>1 repeats the whole computation for
    loop-delta hardware timing (outputs are rewritten identically)."""
    import concourse.bacc as bacc
    import concourse.mybir as mybir
    from concourse.tile import TileContext

    f32 = mybir.dt.float32
    u8 = mybir.dt.uint8
    Alu = mybir.AluOpType
    lif_op = _register_lif_op() if fused else None

    nc = bacc.Bacc("TRN2", target_bir_lowering=False)
    x = nc.dram_tensor("x", [BPC, T, C, HW], f32, kind="ExternalInput")
    y = nc.dram_tensor("y", [BPC, T, C, HW], u8, kind="ExternalOutput")

    with TileContext(nc) as tc:
        with (
            tc.tile_pool(name="xp", bufs=2) as xp,
            tc.tile_pool(name="up", bufs=3) as up,
            tc.tile_pool(name="rp", bufs=2) as rp,
            tc.tile_pool(name="yp", bufs=2) as yp,
            tc.tile_pool(name="cp", bufs=1) as cp,
        ):
            neg_thresh = cp.tile([C, 1], f32)
            nc.vector.memset(neg_thresh[:], -THRESH)
            for _rep in range(reps):
                for b in range(BPC):
                    xb = xp.tile([C, T, HW], f32, tag="xb")
                    nc.sync.dma_start(xb[:], x[b].rearrange("t c w -> c t w"))
                    yb = yp.tile([C, T, HW], u8, tag="yb")
                    u = None  # u_{t-1} AP
                    r = None  # (flat variant) reset state
                    for t in range(T):
                        xt = xb[:, t, :]
                        if t == 0:
                            u = xt
                        elif fused:
                            un = up.tile([C, HW], f32, tag="u")
                            nc.vector._custom_dve(
                                lif_op,
                                out=un[:],
                                in0=u if t == 1 else u[:],
                                in1=xt,
                                s0=THRESH,
                                s1=TAU,
                            )
                            u = un
                        else:
                            un = up.tile([C, HW], f32, tag="u")
                            nc.vector.scalar_tensor_tensor(
                                un[:], r[:], TAU, xt, Alu.mult, Alu.add
                            )
                            u = un
                        uin = u if t == 0 else u[:]
                        nc.scalar.activation(
                            yb[:, t, :],
                            uin,
                            mybir.ActivationFunctionType.Sign,
                            bias=neg_thresh[:],
                        )
                        if not fused and t < T - 1:
                            rn = rp.tile([C, HW], f32, tag="r")
                            nc.vector.scalar_tensor_tensor(
                                rn[:], uin, THRESH, uin, Alu.is_lt, Alu.mult
                            )
                            r = rn
                    # store rides the second HWDGE ring (ACT), loads on SP
                    nc.scalar.dma_start(y[b].rearrange("t c w -> c t w"), yb[:])
    nc.compile()
    return nc


def _get_nc():
    variant = os.environ.get("LIF_VARIANT", "fused")
    if variant not in _nc_cache:
        _nc_cache[variant] = build_bass(fused=variant == "fused")
    return _nc_cache[variant]


def kernel(x):
    global LAST_RESULTS
    from concourse import bass_utils

    assert x.shape == (B, T, C, H, W) and x.dtype == np.float32
    xs = np.ascontiguousarray(x.reshape(B, T, C, HW))
    nc = _get_nc()
    in_maps = [
        {"x": np.ascontiguousarray(xs[i * BPC : (i + 1) * BPC])}
        for i in range(N_CORES)
    ]
    res = bass_utils.run_bass_kernel_spmd(
        nc,
        in_maps,
        core_ids=list(range(N_CORES)),
        trace=bool(int(os.environ.get("LIF_TRACE", "0"))),
    )
    LAST_RESULTS = res
    out = np.empty((B, T, C, HW), dtype=np.float32)
    for i in range(N_CORES):
        # ACT Sign gives {-1,0,+1}; in uint8 that is {255,0,1}: spike == 1
        out[i * BPC : (i + 1) * BPC] = res.results[i]["y"] == 1
    return out.reshape(B, T, C, HW).reshape(B, T, C, H, W)


# revision 11
# speedup vs baseline: 20.3962x; 1.0943x over previous
"""LIF spike kernel for Trainium2 (Bass/Tile), data-parallel over 8 NeuronCores.

Problem: x [32, 8, 128, 32, 32] fp32 -> spikes [32, 8, 128, 32, 32] fp32
    mem_t = mem_{t-1} * 0.25 + x_t ; spike = (mem >= 0.5) ; mem *= (1 - spike)

Sharding: batch dim (32) split 4-per-core across 8 cores; no cross-core comm.

Per-core device program (x shaped [4, 8, 128, 1024] fp32, y uint8):
  - The LIF update is elementwise, so no transpose is needed anywhere.
    Per batch b, ONE 4 MiB dma_start loads x[b] into an SBUF tile
    [128, 8*1024] via "t c w -> c (t w)" — per-partition chunks are the
    contiguous 4 KiB w-rows, so the DMA runs near peak HBM bandwidth
    (the old "b c w -> c b w" per-step loads were the documented slow
    rearrange path and dominated the runtime).
  - per time step t on [128, 1024] slices:
        u_t = select(u_{t-1} < 0.5, TAU*u_{t-1}, 0) + x_t
              -- ONE fused custom-DVE op (registered below): decay,
                 reset and input accumulation in a single DVE pass.
        y_t = Sign(u_t - 0.5) on ACT -> uint8 saturates {-1,0,1}->{0,0,1}
  - spikes accumulate in a [128, 8*1024] uint8 tile; ONE 1 MiB store per b,
    written c-major ([b, c, t, w]) so every store descriptor is an 8 KiB
    contiguous run; the host moveaxis restores [b, t, c, w].
  - host maps uint8 (spike == 1) back to fp32 (exact).
All arithmetic is fp32 and rounds identically to the jax reference
(mult by 0.25 is exact; select/compare exact; a single rounding add per
step), so the spike train matches bitwise.
"""

import os
import numpy as np

B, T, C, H, W = 32, 8, 128, 32, 32
HW = H * W
N_CORES = 8
BPC = B // N_CORES  # batches per core
TAU = 0.25
THRESH = 0.5

_nc_cache = {}
LAST_RESULTS = None
_LIF_OP = None


def _register_lif_op():
    """Register the fused LIF-step custom DVE op with concourse's runtime
    table (the documented extension point is appending to dve_ops.OPS).

        out = select(in0 < s0, in0 * s1, 0) + in1
            = TAU*u*[u < THRESH] + x      (s0=THRESH, s1=TAU)

    One DVE instruction per time step instead of two scalar_tensor_tensor
    passes; exact fp32 (mult by 2^-2 exact, one rounding add)."""
    global _LIF_OP
    if _LIF_OP is not None:
        return _LIF_OP
    from concourse import dve_ops
    from concourse.dve_spec import (
        Spec,
        Src0,
        Src1,
        C0,
        C1,
        Zero,
        select,
        lower,
        _has_src1,
    )
    from concourse.dve_uop import DveOpSpec

    name = "LIF_STEP_ANT"
    for op in dve_ops.OPS:
        if op.name == name:
            _LIF_OP = op
            return op

    body = select(Src0 < C0, Src0 * C1, Zero) + Src1

    def _ref(in0, in1, s0, s1, imm2):
        return (
            np.where(in0 < s0, in0.astype(np.float32) * s1, 0.0).astype(np.float32)
            + in1
        )

    spec = Spec(body=body, reference=_ref)
    opcode = dve_ops._CUSTOM_DVE_ROW_BASE + len(dve_ops.OPS)
    assert opcode < 0x20
    shas = {}
    for ver in ("v3", "v4"):
        try:
            uops = lower(spec, ver=ver)
        except Exception:
            continue
        shas[ver] = DveOpSpec(
            name=name, opcode=opcode, uops=uops, rd1_en=_has_src1(spec)
        ).sha(ver)
    op = dve_ops.DveOp(name, spec, subdim=False, uops_sha=shas)
    dve_ops.OPS.append(op)
    dve_ops._SUB_OPCODE_FOR_NAME[name] = opcode
    dve_ops.CUSTOM_DVE_SPECS[name] = spec
    _LIF_OP = op
    return op


def build_bass(reps=1, fused=True):
    """Per-core Bass program. reps>1 repeats the whole computation for
    loop-delta hardware timing (outputs are rewritten identically)."""
    import concourse.bacc as bacc
    import concourse.mybir as mybir
    from concourse.tile import TileContext

    f32 = mybir.dt.float32
    u8 = mybir.dt.uint8
    Alu = mybir.AluOpType
    lif_op = _register_lif_op() if fused else None

    nc = bacc.Bacc("TRN2", target_bir_lowering=False)
    x = nc.dram_tensor("x", [BPC, T, C, HW], f32, kind="ExternalInput")
    # y is c-major ([b, c, t, w]) so the store is fully contiguous per
    # partition (8 KiB runs instead of 1 KiB) — the host moveaxis during
    # unshard puts t back in front of c.
    y = nc.dram_tensor("y", [BPC, C, T, HW], u8, kind="ExternalOutput")

    with TileContext(nc) as tc:
        with (
            tc.tile_pool(name="xp", bufs=3) as xp,
            tc.tile_pool(name="up", bufs=3) as up,
            tc.tile_pool(name="rp", bufs=2) as rp,
            tc.tile_pool(name="yp", bufs=2) as yp,
            tc.tile_pool(name="cp", bufs=1) as cp,
        ):
            neg_thresh = cp.tile([C, 1], f32)
            nc.vector.memset(neg_thresh[:], -THRESH)
            for _rep in range(reps):
                for b in range(BPC):
                    xb = xp.tile([C, T, HW], f32, tag="xb")
                    nc.sync.dma_start(xb[:], x[b].rearrange("t c w -> c t w"))
                    yb = yp.tile([C, T, HW], u8, tag="yb")
                    u = None  # u_{t-1} AP
                    r = None  # (flat variant) reset state
                    for t in range(T):
                        xt = xb[:, t, :]
                        if t == 0:
                            u = xt
                        elif fused:
                            un = up.tile([C, HW], f32, tag="u")
                            nc.vector._custom_dve(
                                lif_op,
                                out=un[:],
                                in0=u if t == 1 else u[:],
                                in1=xt,
                                s0=THRESH,
                                s1=TAU,
                            )
                            u = un
                        else:
                            un = up.tile([C, HW], f32, tag="u")
                            nc.vector.scalar_tensor_tensor(
                                un[:], r[:], TAU, xt, Alu.mult, Alu.add
                            )
                            u = un
                        uin = u if t == 0 else u[:]
                        nc.scalar.activation(
                            yb[:, t, :],
                            uin,
                            mybir.ActivationFunctionType.Sign,
                            bias=neg_thresh[:],
                        )
                        if not fused and t < T - 1:
                            rn = rp.tile([C, HW], f32, tag="r")
                            nc.vector.scalar_tensor_tensor(
                                rn[:], uin, THRESH, uin, Alu.is_lt, Alu.mult
                            )
                            r = rn
                    # store rides the second HWDGE ring (ACT), loads on SP
                    nc.scalar.dma_start(y[b], yb[:])
    nc.compile()
    return nc


def _get_nc():
    variant = os.environ.get("LIF_VARIANT", "fused")
    if variant not in _nc_cache:
        _nc_cache[variant] = build_bass(fused=variant == "fused")
    return _nc_cache[variant]


def kernel(x):
    global LAST_RESULTS
    from concourse import bass_utils

    assert x.shape == (B, T, C, H, W) and x.dtype == np.float32
    xs = np.ascontiguousarray(x.reshape(B, T, C, HW))
    nc = _get_nc()
    in_maps = [
        {"x": np.ascontiguousarray(xs[i * BPC : (i + 1) * BPC])}
        for i in range(N_CORES)
    ]
    res = bass_utils.run_bass_kernel_spmd(
        nc,
        in_maps,
        core_ids=list(range(N_CORES)),
        trace=bool(int(os.environ.get("LIF_TRACE", "0"))),
    )
    LAST_RESULTS = res
    out = np.empty((B, T, C, HW), dtype=np.float32)
    for i in range(N_CORES):
        # ACT Sign gives {-1,0,+1}; in uint8 that is {255,0,1}: spike == 1.
        # y arrives c-major [BPC, C, T, HW]; moveaxis restores [BPC, T, C, HW].
        yi = np.moveaxis(res.results[i]["y"], 1, 2)
        out[i * BPC : (i + 1) * BPC] = yi == 1
    return out.reshape(B, T, C, H, W)


# revision 15
# speedup vs baseline: 20.9717x; 1.0282x over previous
"""LIF spike kernel for Trainium2 (Bass/Tile), data-parallel over 8 NeuronCores.

Problem: x [32, 8, 128, 32, 32] fp32 -> spikes [32, 8, 128, 32, 32] fp32
    mem_t = mem_{t-1} * 0.25 + x_t ; spike = (mem >= 0.5) ; mem *= (1 - spike)

Sharding: batch dim (32) split 4-per-core across 8 cores; no cross-core comm.

Per-core device program (x shaped [4, 8, 128, 1024] fp32, y uint8):
  - The LIF update is elementwise, so no transpose is needed anywhere.
    The host shards each core's x to c-major [4, 128, 8, 1024] (the
    per-shard copy happens regardless); per batch b, ONE fully
    contiguous 4 MiB dma_start loads x[b] into an SBUF tile
    [128, 8, 1024] at peak HBM bandwidth (the old "b c w -> c b w"
    per-step loads were the documented slow rearrange path and
    dominated the runtime).
  - per time step t on [128, 1024] slices:
        u_t = select(u_{t-1} < 0.5, TAU*u_{t-1}, 0) + x_t
              -- ONE fused custom-DVE op (registered below): decay,
                 reset and input accumulation in a single DVE pass.
        y_t = Sign(u_t - 0.5) on ACT -> uint8 saturates {-1,0,1}->{0,0,1}
  - spikes accumulate in a [128, 8*1024] uint8 tile; ONE 1 MiB store per b,
    written c-major ([b, c, t, w]) so every store descriptor is an 8 KiB
    contiguous run; the host moveaxis restores [b, t, c, w].
  - host maps uint8 (spike == 1) back to fp32 (exact).
All arithmetic is fp32 and rounds identically to the jax reference
(mult by 0.25 is exact; select/compare exact; a single rounding add per
step), so the spike train matches bitwise.
"""

import os
import numpy as np

B, T, C, H, W = 32, 8, 128, 32, 32
HW = H * W
N_CORES = 8
BPC = B // N_CORES  # batches per core
TAU = 0.25
THRESH = 0.5

_nc_cache = {}
LAST_RESULTS = None
_LIF_OP = None


def _register_lif_op():
    """Register the fused LIF-step custom DVE op with concourse's runtime
    table (the documented extension point is appending to dve_ops.OPS).

        out = select(in0 < s0, in0 * s1, 0) + in1
            = TAU*u*[u < THRESH] + x      (s0=THRESH, s1=TAU)

    One DVE instruction per time step instead of two scalar_tensor_tensor
    passes; exact fp32 (mult by 2^-2 exact, one rounding add)."""
    global _LIF_OP
    if _LIF_OP is not None:
        return _LIF_OP
    from concourse import dve_ops
    from concourse.dve_spec import (
        Spec,
        Src0,
        Src1,
        C0,
        C1,
        Zero,
        select,
        lower,
        _has_src1,
    )
    from concourse.dve_uop import DveOpSpec

    name = "LIF_STEP_ANT"
    for op in dve_ops.OPS:
        if op.name == name:
            _LIF_OP = op
            return op

    body = select(Src0 < C0, Src0 * C1, Zero) + Src1

    def _ref(in0, in1, s0, s1, imm2):
        return (
            np.where(in0 < s0, in0.astype(np.float32) * s1, 0.0).astype(np.float32)
            + in1
        )

    spec = Spec(body=body, reference=_ref)
    opcode = dve_ops._CUSTOM_DVE_ROW_BASE + len(dve_ops.OPS)
    assert opcode < 0x20
    shas = {}
    for ver in ("v3", "v4"):
        try:
            uops = lower(spec, ver=ver)
        except Exception:
            continue
        shas[ver] = DveOpSpec(
            name=name, opcode=opcode, uops=uops, rd1_en=_has_src1(spec)
        ).sha(ver)
    op = dve_ops.DveOp(name, spec, subdim=False, uops_sha=shas)
    dve_ops.OPS.append(op)
    dve_ops._SUB_OPCODE_FOR_NAME[name] = opcode
    dve_ops.CUSTOM_DVE_SPECS[name] = spec
    _LIF_OP = op
    return op


def build_bass(reps=1, fused=True):
    """Per-core Bass program. reps>1 repeats the whole computation for
    loop-delta hardware timing (outputs are rewritten identically)."""
    import concourse.bacc as bacc
    import concourse.mybir as mybir
    from concourse.tile import TileContext

    f32 = mybir.dt.float32
    u8 = mybir.dt.uint8
    Alu = mybir.AluOpType
    lif_op = _register_lif_op() if fused else None

    nc = bacc.Bacc("TRN2", target_bir_lowering=False)
    # Both tensors are c-major ([b, c, t, w]) so every DMA descriptor is a
    # long contiguous per-partition run (32 KiB loads / 8 KiB stores). The
    # host moves the t axis during shard/unshard — the per-core shard copy
    # pays one ascontiguousarray either way, so this is free on the host.
    x = nc.dram_tensor("x", [BPC, C, T, HW], f32, kind="ExternalInput")
    y = nc.dram_tensor("y", [BPC, C, T, HW], u8, kind="ExternalOutput")

    with TileContext(nc) as tc:
        with (
            tc.tile_pool(name="xp", bufs=3) as xp,
            tc.tile_pool(name="up", bufs=3) as up,
            tc.tile_pool(name="rp", bufs=2) as rp,
            tc.tile_pool(name="yp", bufs=2) as yp,
            tc.tile_pool(name="cp", bufs=1) as cp,
        ):
            neg_thresh = cp.tile([C, 1], f32)
            nc.vector.memset(neg_thresh[:], -THRESH)
            for _rep in range(reps):
                for b in range(BPC):
                    xb = xp.tile([C, T, HW], f32, tag="xb")
                    nc.sync.dma_start(xb[:], x[b])
                    yb = yp.tile([C, T, HW], u8, tag="yb")
                    u = None  # u_{t-1} AP
                    r = None  # (flat variant) reset state
                    for t in range(T):
                        xt = xb[:, t, :]
                        if t == 0:
                            u = xt
                        elif fused:
                            un = up.tile([C, HW], f32, tag="u")
                            nc.vector._custom_dve(
                                lif_op,
                                out=un[:],
                                in0=u if t == 1 else u[:],
                                in1=xt,
                                s0=THRESH,
                                s1=TAU,
                            )
                            u = un
                        else:
                            un = up.tile([C, HW], f32, tag="u")
                            nc.vector.scalar_tensor_tensor(
                                un[:], r[:], TAU, xt, Alu.mult, Alu.add
                            )
                            u = un
                        uin = u if t == 0 else u[:]
                        nc.scalar.activation(
                            yb[:, t, :],
                            uin,
                            mybir.ActivationFunctionType.Sign,
                            bias=neg_thresh[:],
                        )
                        if not fused and t < T - 1:
                            rn = rp.tile([C, HW], f32, tag="r")
                            nc.vector.scalar_tensor_tensor(
                                rn[:], uin, THRESH, uin, Alu.is_lt, Alu.mult
                            )
                            r = rn
                    # store rides the second HWDGE ring (ACT), loads on SP
                    nc.scalar.dma_start(y[b], yb[:])
    nc.compile()
    return nc


def _get_nc():
    variant = os.environ.get("LIF_VARIANT", "fused")
    if variant not in _nc_cache:
        _nc_cache[variant] = build_bass(fused=variant == "fused")
    return _nc_cache[variant]


def kernel(x):
    global LAST_RESULTS
    from concourse import bass_utils

    assert x.shape == (B, T, C, H, W) and x.dtype == np.float32
    # shard to per-core c-major [BPC, C, T, HW] (the copy happens anyway)
    xs = np.moveaxis(x.reshape(B, T, C, HW), 1, 2)
    nc = _get_nc()
    in_maps = [
        {"x": np.ascontiguousarray(xs[i * BPC : (i + 1) * BPC])}
        for i in range(N_CORES)
    ]
    res = bass_utils.run_bass_kernel_spmd(
        nc,
        in_maps,
        core_ids=list(range(N_CORES)),
        trace=bool(int(os.environ.get("LIF_TRACE", "0"))),
    )
    LAST_RESULTS = res
    out = np.empty((B, T, C, HW), dtype=np.float32)
    for i in range(N_CORES):
        # ACT Sign gives {-1,0,+1}; in uint8 that is {255,0,1}: spike == 1.
        # y arrives c-major [BPC, C, T, HW]; moveaxis restores [BPC, T, C, HW].
        yi = np.moveaxis(res.results[i]["y"], 1, 2)
        out[i * BPC : (i + 1) * BPC] = yi == 1
    return out.reshape(B, T, C, H, W)
